# revision 29
# baseline (speedup 1.0000x reference)
"""BrainTumorGCNN Trainium2 kernel — v2: k-sharded dense classifier.

Strategy (8 cores, SPMD):
  - Core c owns batch c's GCN end-to-end (A^T resident in SBUF as fp8,
    DoubleRow fp8 pair-matmuls halve PE time on the two A-contractions).
  - Dense classifier is k-sharded: instead of every core streaming the
    full 33.5MB Wd (the v1 bottleneck — 8x redundant HBM traffic), core c
    streams only rows [c*16384, (c+1)*16384) (4.2MB bf16). The flat
    feature vectors are exchanged with a single in-NEFF AllToAll (262KB
    per rank, mesh, ~6us, runs on TOPSP/SDMA silicon) so core c holds
    shard c of every batch's flat vector. Core c emits partial
    z[d, b] = sum_{k in shard c} Wd[k, d] * flat_b[k] for all 8 batches.
  - Host combine: sum the 8 partial-z tiles [128, 8] and run the tiny
    head (relu(+bd) @ Wo + bo -> sigmoid) in float64 numpy (~2K flops).
    The logits have |z| >= ~680 sign margins, so outputs saturate.
Per-core HBM traffic drops ~38MB -> ~9.5MB.
"""

import numpy as np

import concourse.bacc as bacc
import concourse.mybir as mybir
from concourse import tile

B, N, F, H1, H2, D1 = 8, 2048, 128, 32, 64, 128
NCORES = 8
P = 128
MC = N // P             # 16 contraction chunks of 128
NBLK = N // 512         # 4 node blocks of 512
KTOT = N * H2           # 131072 flat rows of Wd
KS = KTOT // NCORES     # 16384 rows per core
DCH = KS // P           # 128 dense chunks per core
WDT = 4                 # wd shard streamed in 4 SBUF tiles
WDC = DCH // WDT        # 32 chunks per tile

REPLICATED = frozenset({"w1", "w2", "b1", "b2r"})
BF = mybir.dt.bfloat16
NP_BF = mybir.dt.np(BF)
F8 = mybir.dt.float8e4
NP_F8 = mybir.dt.np(F8)
RG = [list(range(NCORES))]

_cache = {}


def _build(chain=1):
    f32 = mybir.dt.float32
    nc = bacc.Bacc("TRN2", target_bir_lowering=False, debug=False,
                   num_devices=NCORES)

    at_ext = nc.dram_tensor("at", [MC, P, N], F8, kind="ExternalInput")
    xt_ext = nc.dram_tensor("xt", [F, N], BF, kind="ExternalInput")
    w1_ext = nc.dram_tensor("w1", [F, H1], BF, kind="ExternalInput")
    w2_ext = nc.dram_tensor("w2", [H1, H2], BF, kind="ExternalInput")
    b1_ext = nc.dram_tensor("b1", [H1, 1], f32, kind="ExternalInput")
    b2r_ext = nc.dram_tensor("b2r", [1, H2], BF, kind="ExternalInput")
    wds_ext = nc.dram_tensor("wds", [WDT, P, WDC * P], BF, kind="ExternalInput")
    out_ext = nc.dram_tensor("out", [D1, 2 * B], f32, kind="ExternalOutput")

    Relu = mybir.ActivationFunctionType.Relu
    Copy = mybir.ActivationFunctionType.Copy
    DR = mybir.MatmulPerfMode.DoubleRow

    with tile.TileContext(nc) as tc:
        with (
            tc.tile_pool(name="const", bufs=1) as cpool,
            tc.tile_pool(name="amat", bufs=2) as apool,
            tc.tile_pool(name="wd", bufs=2) as wdpool,
            tc.tile_pool(name="work", bufs=2) as wpool,
            tc.tile_pool(name="xch", bufs=2) as xpool,
            tc.tile_pool(name="dram", bufs=2, space="DRAM") as dpool,
            tc.tile_pool(name="ps_small", bufs=1, space="PSUM") as ps_s,
            tc.tile_pool(name="ps_agg", bufs=1, space="PSUM") as ps_a,
            tc.tile_pool(name="ps_z", bufs=2, space="PSUM") as ps_z,
        ):
            xt_sb = cpool.tile([F, N], BF)
            nc.sync.dma_start(xt_sb[:], xt_ext[:])
            w1_sb = cpool.tile([F, H1], BF)
            nc.sync.dma_start(w1_sb[:], w1_ext[:])
            w2_sb = cpool.tile([H1, H2], BF)
            nc.sync.dma_start(w2_sb[:], w2_ext[:])
            b1_sb = cpool.tile([H1, 1], f32)
            nc.sync.dma_start(b1_sb[:], b1_ext[:])
            b2r_sb = cpool.tile([1, H2], BF)
            nc.sync.dma_start(b2r_sb[:], b2r_ext[:])
            ones_sb = cpool.tile([1, P], BF)
            nc.vector.memset(ones_sb[:], 1.0)

            for _it in range(chain):
              # ---- A^T chunks resident in SBUF (one HBM read, fp8e4m3),
              #      grouped 8 chunks per DMA to amortize queue overheads ----
              AG = 8
              a_groups = []
              for g in range(MC // AG):
                  a_g = apool.tile([P, AG * N], F8, tag=f"a{g}")
                  (nc.sync if g % 2 == 0 else nc.scalar).dma_start(
                      a_g[:], at_ext[g * AG:(g + 1) * AG])
                  a_groups.append(a_g)

              def a_pair(mc, lo, ln):
                  # [128, 2, ln] view of chunks (mc, mc+1), nodes [lo, lo+ln)
                  g, o = divmod(mc, AG)
                  return (a_groups[g][:]
                          .rearrange("p (c n) -> p c n", c=AG)
                          [:, o:o + 2, lo:lo + ln])

              # ---- wd shard tiles: tile 0 rides the scalar HWDGE queue after
              #      the A groups; tiles 1-3 go to the gpsimd SWDGE queue so
              #      all three DMA queues carry ~3.1MB per iteration ----
              wd_tiles = []
              for t in range(WDT):
                  wd_t = wdpool.tile([P, WDC * P], BF, tag=f"wd{t}")
                  (nc.scalar if t == 0 else nc.gpsimd).dma_start(
                      wd_t[:], wds_ext[t])
                  wd_tiles.append(wd_t)

              # ---- t1 = x @ W1 -> fp8, chunk mc at cols [mc*H1,(mc+1)*H1);
              #      all 16 matmuls land in one PSUM bank, single copy ----
              t1_sb = wpool.tile([P, MC * H1], F8, tag="t1")
              pt1_full = ps_s.tile([P, MC * H2], f32, tag="pt", name="pt1")
              pt1 = pt1_full[:, :MC * H1]
              for mc in range(MC):
                  nc.tensor.matmul(pt1[:, mc * H1:(mc + 1) * H1],
                                   xt_sb[:, mc * P:(mc + 1) * P],
                                   w1_sb[:], start=True, stop=True)
              nc.scalar.activation(t1_sb[:], pt1[:], Copy)
              t1_3 = t1_sb[:].rearrange("p (c h) -> p c h", c=MC)

              # ---- h1^T = relu((A @ t1)^T + b1) : bf16 [H1, N],
              #      DoubleRow fp8 pair-matmuls, two 2-bank PSUM halves ----
              h1t_sb = wpool.tile([H1, N], BF, tag="h1t")
              for half in range(2):
                  pa1 = ps_a.tile([H1, N // 2], f32, tag="pagg", name="pa1")
                  for nb in range(2 * half, 2 * half + 2):
                      for mc in range(0, MC, 2):
                          nc.tensor.matmul(
                              pa1[:, (nb - 2 * half) * 512:
                                  (nb - 2 * half + 1) * 512],
                              t1_3[:, mc:mc + 2, :], a_pair(mc, nb * 512, 512),
                              start=(mc == 0), stop=(mc == MC - 2),
                              perf_mode=DR)
                  nc.scalar.activation(h1t_sb[:, half * 1024:(half + 1) * 1024],
                                       pa1[:], Relu, bias=b1_sb[:])

              # ---- t2 = h1 @ W2 -> fp8, one 2-bank PSUM tile ----
              t2_sb = wpool.tile([P, MC * H2], F8, tag="t2")
              pt2 = ps_s.tile([P, MC * H2], f32, tag="pt")
              for mc in range(MC):
                  nc.tensor.matmul(pt2[:, mc * H2:(mc + 1) * H2],
                                   h1t_sb[:, mc * P:(mc + 1) * P],
                                   w2_sb[:], start=True, stop=True)
              nc.scalar.activation(t2_sb[:], pt2[:], Copy)
              t2_3 = t2_sb[:].rearrange("p (c h) -> p c h", c=MC)

              # ---- flat2 = relu(A @ t2 + b2), FLIPPED: A chunks stationary,
              #      t2 moving -> out [128 nodes, 64 ch] per node chunk; 8x
              #      fewer streamed columns than the channel-major form.
              #      flat2_sb[p, nc2*64 + h] = h2[node nc2*128+p, ch h];
              #      bias enters via a ones-row matmul into each slice ----
              flat_sb = wpool.tile([P, KTOT // P], BF, tag="flat")
              pa2 = ps_a.tile([P, MC * H2], f32, tag="pagg2")
              for nc2 in range(MC):
                  sl = pa2[:, nc2 * H2:(nc2 + 1) * H2]
                  nc.tensor.matmul(sl, ones_sb[:], b2r_sb[:],
                                   start=True, stop=False,
                                   skip_group_check=True)
                  for mc in range(0, MC, 2):
                      nc.tensor.matmul(
                          sl,
                          a_pair(mc, nc2 * P, P), t2_3[:, mc:mc + 2, :],
                          start=False, stop=(mc == MC - 2),
                          perf_mode=DR, skip_group_check=True)
              nc.scalar.activation(flat_sb[:], pa2[:], Relu)

              # ---- AllToAll: shard j of this core's flat -> core j;
              #      receive shard c of every batch's flat ----
              asend = dpool.tile([NCORES, P, P], BF, tag="asend", bufs=2)
              for j in range(NCORES):
                  nc.sync.dma_start(asend[j], flat_sb[:, j * P:(j + 1) * P])
              arecv = dpool.tile([NCORES, P, P], BF, tag="arecv", bufs=2)
              nc.gpsimd.collective_compute(
                  "AllToAll", mybir.AluOpType.bypass,
                  replica_groups=RG,
                  ins=[asend[:].opt()], outs=[arecv[:].opt()])

              # rt[p, b*128 + cc] = flat_b[c*16384 + cc*128 + p]
              rt = xpool.tile([P, NCORES * P], BF, tag="rt", bufs=2)
              for b in range(NCORES):
                  nc.sync.dma_start(rt[:, b * P:(b + 1) * P], arecv[b])
              rt3 = rt[:].rearrange("p (b c) -> p c b", b=NCORES)

              # ---- dense partials, two interleaved accumulation chains:
              #      zp[d, e*B+b] += Wd_chunk[k, d] * rt[k, b]; the host sums
              #      the two chains along with the cross-core partials ----
              zp = ps_z.tile([D1, 2 * B], f32)
              for ci in range(DCH):
                  t, o = divmod(ci, WDC)
                  e = ci % 2
                  nc.tensor.matmul(
                      zp[:, e * B:(e + 1) * B],
                      wd_tiles[t][:, o * P:(o + 1) * P],
                      rt3[:, ci, :],
                      start=(ci < 2), stop=(ci >= DCH - 2),
                  )
              o_sb = wpool.tile([D1, 2 * B], f32, tag="osb")
              nc.scalar.activation(o_sb[:], zp[:], Copy)
              nc.gpsimd.dma_start(out_ext[:], o_sb[:])

    nc.compile()
    return nc


def _make_runner_for(nc):
    return _runner_from_nc(nc)


def _get_runner(chain=1):
    """Cached jitted shard_map executable around the Bass NEFF. chain>1
    repeats the kernel body inside the NEFF for wall-clock timing."""
    key = ("runner", chain)
    if key in _cache:
        return _cache[key]
    nckey = ("nc", chain)
    nc = _cache.get(nckey)
    if nc is None:
        nc = _cache[nckey] = _build(chain)
    runner = _runner_from_nc(nc)
    _cache[key] = runner
    return runner


def _runner_from_nc(nc):
    import jax
    from jax.experimental.shard_map import shard_map
    from jax.sharding import Mesh, PartitionSpec, NamedSharding
    from concourse import bass2jax
    bass2jax.install_neuronx_cc_hook()

    partition_name = nc.partition_id_tensor.name if nc.partition_id_tensor else None
    in_names, out_names, out_avals, zero_outs = [], [], [], []
    for alloc in nc.m.functions[0].allocations:
        if not isinstance(alloc, mybir.MemoryLocationSet):
            continue
        name = alloc.memorylocations[0].name
        if alloc.kind == "ExternalInput":
            if name != partition_name:
                in_names.append(name)
        elif alloc.kind == "ExternalOutput":
            shape = tuple(alloc.tensor_shape)
            dtype = mybir.dt.np(alloc.dtype)
            out_names.append(name)
            out_avals.append(jax.core.ShapedArray(shape, dtype))
            zero_outs.append(np.zeros(shape, dtype))
    n_params = len(in_names)
    n_outs = len(out_avals)
    all_names = in_names + out_names + ([partition_name] if partition_name else [])
    donate = tuple(range(n_params, n_params + n_outs))

    def _body(*args):
        operands = list(args)
        if partition_name is not None:
            operands.append(bass2jax.partition_id_tensor())
        return tuple(bass2jax._bass_exec_p.bind(
            *operands,
            out_avals=tuple(out_avals),
            in_names=tuple(all_names),
            out_names=tuple(out_names),
            lowering_input_output_aliases=(),
            sim_require_finite=True,
            sim_require_nnan=True,
            nc=nc,
        ))

    devices = jax.devices()[:NCORES]
    mesh = Mesh(np.asarray(devices), ("core",))
    in_specs = tuple(
        PartitionSpec() if name in REPLICATED else PartitionSpec("core")
        for name in in_names) + (PartitionSpec("core"),) * n_outs
    fn = jax.jit(
        shard_map(_body, mesh=mesh, in_specs=in_specs,
                  out_specs=(PartitionSpec("core"),) * n_outs,
                  check_rep=False),
        donate_argnums=donate, keep_unused=True,
    )
    shardings = {
        name: NamedSharding(mesh, PartitionSpec() if name in REPLICATED
                            else PartitionSpec("core"))
        for name in in_names}
    return {
        "fn": fn, "in_names": in_names, "out_names": out_names,
        "zero_outs": zero_outs, "mesh": mesh,
        "sharding": NamedSharding(mesh, PartitionSpec("core")),
        "shardings": shardings,
        "out_avals": out_avals,
    }


def _prep(x, a, W1, b1, W2, b2, Wd, bd, Wo, bo):
    """Host-side shard/layout prep -> dict of concatenated (8*dim0) inputs."""
    x = np.asarray(x, np.float32)
    a = np.asarray(a, np.float32)
    W1 = np.ascontiguousarray(np.asarray(W1, NP_BF))
    W2 = np.ascontiguousarray(np.asarray(W2, NP_BF))
    b1c = np.asarray(b1, np.float32).reshape(H1, 1)
    b2r = np.ascontiguousarray(np.asarray(b2, np.float32)
                               .reshape(1, H2).astype(NP_BF))
    Wd = np.asarray(Wd, np.float32)

    at = np.ascontiguousarray(
        a.astype(NP_F8).transpose(0, 2, 1)).reshape(NCORES * MC, P, N)
    xt = np.ascontiguousarray(
        x.astype(NP_BF).transpose(0, 2, 1)).reshape(NCORES * F, N)
    # flat ordering on device is (node_chunk nc2, channel h, node_in_chunk p);
    # permute Wd rows k = (nc2*128+p)*64 + h to match, then tile per core:
    # wds[c, t, p, ci*128 + d] = Wd_perm[c*16384 + (t*WDC + ci)*128 + p, d]
    wds = np.ascontiguousarray(
        Wd.astype(NP_BF).reshape(MC, P, H2, D1)
        .transpose(0, 2, 1, 3)                       # [nc2, h, p, d]
        .reshape(NCORES, WDT, WDC, P, D1)            # [c, t, ci, p, d]
        .transpose(0, 1, 3, 2, 4)
        .reshape(NCORES * WDT, P, WDC * P))

    return {
        "at": at, "xt": xt, "w1": W1, "w2": W2, "b1": b1c,
        "b2r": b2r, "wds": wds,
    }


def _run(runner, concat_ins):
    args = [concat_ins[name] for name in runner["in_names"]]
    zeros = [np.zeros((NCORES * z.shape[0], *z.shape[1:]), z.dtype)
             for z in runner["zero_outs"]]
    return runner["fn"](*args, *zeros)


def kernel(x, a, W1, b1, W2, b2, Wd, bd, Wo, bo):
    runner = _get_runner()
    concat_ins = _prep(x, a, W1, b1, W2, b2, Wd, bd, Wo, bo)
    outs = _run(runner, concat_ins)
    oi = runner["out_names"].index("out")
    partials = np.asarray(outs[oi]).reshape(NCORES, D1, 2, B).astype(np.float64)
    z = partials.sum(axis=(0, 2)).T                # [B, D1]
    hd = np.maximum(z + np.asarray(bd, np.float64).reshape(1, D1), 0.0)
    logit = hd @ np.asarray(Wo, np.float64).reshape(D1, 1) \
        + np.asarray(bo, np.float64).reshape(1, 1)     # [B, 1]
    out = np.where(logit >= 0, 1.0 / (1.0 + np.exp(-np.clip(logit, 0, None))),
                   np.exp(np.clip(logit, None, 0))
                   / (1.0 + np.exp(np.clip(logit, None, 0))))
    return out.astype(np.float32)


# revision 36
# speedup vs baseline: 1.6387x; 1.6387x over previous
"""BrainTumorGCNN Trainium2 kernel — v2: k-sharded dense classifier.

Strategy (8 cores, SPMD):
  - Core c owns batch c's GCN end-to-end (A^T resident in SBUF as fp8,
    DoubleRow fp8 pair-matmuls halve PE time on the two A-contractions).
  - Dense classifier is k-sharded: instead of every core streaming the
    full 33.5MB Wd (the v1 bottleneck — 8x redundant HBM traffic), core c
    streams only rows [c*16384, (c+1)*16384) (4.2MB bf16). The flat
    feature vectors are exchanged with a single in-NEFF AllToAll (262KB
    per rank, mesh, ~6us, runs on TOPSP/SDMA silicon) so core c holds
    shard c of every batch's flat vector. Core c emits partial
    z[d, b] = sum_{k in shard c} Wd[k, d] * flat_b[k] for all 8 batches.
  - Host combine: sum the 8 partial-z tiles [128, 8] and run the tiny
    head (relu(+bd) @ Wo + bo -> sigmoid) in float64 numpy (~2K flops).
    The logits have |z| >= ~680 sign margins, so outputs saturate.
Per-core HBM traffic drops ~38MB -> ~9.5MB.
"""

import numpy as np

import concourse.bacc as bacc
import concourse.mybir as mybir
from concourse import tile

B, N, F, H1, H2, D1 = 8, 2048, 128, 32, 64, 128
NCORES = 8
P = 128
MC = N // P             # 16 contraction chunks of 128
NBLK = N // 512         # 4 node blocks of 512
KTOT = N * H2           # 131072 flat rows of Wd
KS = KTOT // NCORES     # 16384 rows per core
DCH = KS // P           # 128 dense chunks per core
WDT = 4                 # wd shard streamed in 4 SBUF tiles
WDC = DCH // WDT        # 32 chunks per tile

REPLICATED = frozenset({"w1", "w2", "b1", "b2"})
BF = mybir.dt.bfloat16
NP_BF = mybir.dt.np(BF)
F8 = mybir.dt.float8e4
NP_F8 = mybir.dt.np(F8)
RG = [list(range(NCORES))]

_cache = {}


def _build(chain=1):
    f32 = mybir.dt.float32
    nc = bacc.Bacc("TRN2", target_bir_lowering=False, debug=False,
                   num_devices=NCORES)

    at_ext = nc.dram_tensor("at", [MC, P, N], F8, kind="ExternalInput")
    xt_ext = nc.dram_tensor("xt", [F, N], BF, kind="ExternalInput")
    w1_ext = nc.dram_tensor("w1", [F, H1], BF, kind="ExternalInput")
    w2_ext = nc.dram_tensor("w2", [H1, H2], BF, kind="ExternalInput")
    b1_ext = nc.dram_tensor("b1", [H1, 1], f32, kind="ExternalInput")
    b2_ext = nc.dram_tensor("b2", [H2, 1], f32, kind="ExternalInput")
    wds_ext = nc.dram_tensor("wds", [WDT, P, WDC * P], BF, kind="ExternalInput")
    out_ext = nc.dram_tensor("out", [D1, 2 * B], f32, kind="ExternalOutput")

    Relu = mybir.ActivationFunctionType.Relu
    Copy = mybir.ActivationFunctionType.Copy
    DR = mybir.MatmulPerfMode.DoubleRow

    with tile.TileContext(nc) as tc:
        with (
            tc.tile_pool(name="const", bufs=1) as cpool,
            tc.tile_pool(name="amat", bufs=2) as apool,
            tc.tile_pool(name="wd", bufs=2) as wdpool,
            tc.tile_pool(name="work", bufs=2) as wpool,
            tc.tile_pool(name="xch", bufs=2) as xpool,
            tc.tile_pool(name="dram", bufs=2, space="DRAM") as dpool,
            tc.tile_pool(name="ps_small", bufs=1, space="PSUM") as ps_s,
            tc.tile_pool(name="ps_agg", bufs=1, space="PSUM") as ps_a,
            tc.tile_pool(name="ps_z", bufs=2, space="PSUM") as ps_z,
        ):
            xt_sb = cpool.tile([F, N], BF)
            nc.sync.dma_start(xt_sb[:], xt_ext[:])
            w1_sb = cpool.tile([F, H1], BF)
            nc.sync.dma_start(w1_sb[:], w1_ext[:])
            w2_sb = cpool.tile([H1, H2], BF)
            nc.sync.dma_start(w2_sb[:], w2_ext[:])
            b1_sb = cpool.tile([H1, 1], f32)
            nc.sync.dma_start(b1_sb[:], b1_ext[:])
            b2_sb = cpool.tile([H2, 1], f32)
            nc.sync.dma_start(b2_sb[:], b2_ext[:])

            for _it in range(chain):
              # ---- A^T chunks resident in SBUF (one HBM read, fp8e4m3),
              #      grouped 8 chunks per DMA to amortize queue overheads ----
              AG = 8
              a_groups = []
              for g in range(MC // AG):
                  a_g = apool.tile([P, AG * N], F8, tag=f"a{g}")
                  (nc.sync if g % 2 == 0 else nc.scalar).dma_start(
                      a_g[:], at_ext[g * AG:(g + 1) * AG])
                  a_groups.append(a_g)

              def a_pair(mc, lo, ln):
                  # [128, 2, ln] view of chunks (mc, mc+1), nodes [lo, lo+ln)
                  g, o = divmod(mc, AG)
                  return (a_groups[g][:]
                          .rearrange("p (c n) -> p c n", c=AG)
                          [:, o:o + 2, lo:lo + ln])

              # ---- wd shard tiles: tile 0 rides the scalar HWDGE queue after
              #      the A groups; tiles 1-3 go to the gpsimd SWDGE queue so
              #      all three DMA queues carry ~3.1MB per iteration ----
              wd_tiles = []
              for t in range(WDT):
                  wd_t = wdpool.tile([P, WDC * P], BF, tag=f"wd{t}")
                  (nc.scalar if t == 0 else nc.gpsimd).dma_start(
                      wd_t[:], wds_ext[t])
                  wd_tiles.append(wd_t)

              # ---- t1 = x @ W1 -> fp8, chunk mc at cols [mc*H1,(mc+1)*H1);
              #      all 16 matmuls land in one PSUM bank, single copy ----
              t1_sb = wpool.tile([P, MC * H1], F8, tag="t1")
              pt1_full = ps_s.tile([P, MC * H2], f32, tag="pt", name="pt1")
              pt1 = pt1_full[:, :MC * H1]
              for mc in range(MC):
                  nc.tensor.matmul(pt1[:, mc * H1:(mc + 1) * H1],
                                   xt_sb[:, mc * P:(mc + 1) * P],
                                   w1_sb[:], start=True, stop=True)
              nc.scalar.activation(t1_sb[:], pt1[:], Copy)
              t1_3 = t1_sb[:].rearrange("p (c h) -> p c h", c=MC)

              # ---- h1^T = relu((A @ t1)^T + b1) : bf16 [H1, N],
              #      DoubleRow fp8 pair-matmuls, one 4-bank PSUM tile ----
              h1t_sb = wpool.tile([H1, N], BF, tag="h1t")
              pa1_full = ps_a.tile([H2, N], f32, tag="pagg", name="pa1")
              pa1 = pa1_full[:H1, :]
              for nb in range(NBLK):
                  for mc in range(0, MC, 2):
                      nc.tensor.matmul(
                          pa1[:, nb * 512:(nb + 1) * 512],
                          t1_3[:, mc:mc + 2, :], a_pair(mc, nb * 512, 512),
                          start=(mc == 0), stop=(mc == MC - 2),
                          perf_mode=DR)
              nc.scalar.activation(h1t_sb[:], pa1[:], Relu, bias=b1_sb[:])

              # ---- t2 = h1 @ W2 -> fp8, one 2-bank PSUM tile ----
              t2_sb = wpool.tile([P, MC * H2], F8, tag="t2")
              pt2 = ps_s.tile([P, MC * H2], f32, tag="pt")
              for mc in range(MC):
                  nc.tensor.matmul(pt2[:, mc * H2:(mc + 1) * H2],
                                   h1t_sb[:, mc * P:(mc + 1) * P],
                                   w2_sb[:], start=True, stop=True)
              nc.scalar.activation(t2_sb[:], pt2[:], Copy)
              t2_3 = t2_sb[:].rearrange("p (c h) -> p c h", c=MC)

              # ---- flat = relu(A @ t2 + b2) -> bf16 [P, 1024];
              #      column kc holds flat[128*kc : 128*kc+128] ----
              flat_sb = wpool.tile([P, KTOT // P], BF, tag="flat")
              pa2 = ps_a.tile([H2, N], f32, tag="pagg")
              for nb in range(NBLK):
                  for mc in range(0, MC, 2):
                      nc.tensor.matmul(
                          pa2[:, nb * 512:(nb + 1) * 512],
                          t2_3[:, mc:mc + 2, :], a_pair(mc, nb * 512, 512),
                          start=(mc == 0), stop=(mc == MC - 2),
                          perf_mode=DR)
              pv = pa2[:].rearrange("c (f two) -> c two f", two=2)
              nc.scalar.activation(flat_sb[0:H2, :], pv[:, 0, :],
                                   Relu, bias=b2_sb[:])
              nc.scalar.activation(flat_sb[H2:P, :], pv[:, 1, :],
                                   Relu, bias=b2_sb[:])

              # ---- AllToAll: shard j of this core's flat -> core j;
              #      receive shard c of every batch's flat ----
              asend = dpool.tile([NCORES, P, P], BF, tag="asend", bufs=2)
              for j in range(NCORES):
                  nc.sync.dma_start(asend[j], flat_sb[:, j * P:(j + 1) * P])
              arecv = dpool.tile([NCORES, P, P], BF, tag="arecv", bufs=2)
              nc.gpsimd.collective_compute(
                  "AllToAll", mybir.AluOpType.bypass,
                  replica_groups=RG,
                  ins=[asend[:].opt()], outs=[arecv[:].opt()])

              # rt[p, b*128 + cc] = flat_b[c*16384 + cc*128 + p]
              rt = xpool.tile([P, NCORES * P], BF, tag="rt", bufs=2)
              for b in range(NCORES):
                  nc.sync.dma_start(rt[:, b * P:(b + 1) * P], arecv[b])
              rt3 = rt[:].rearrange("p (b c) -> p c b", b=NCORES)

              # ---- dense partials, two interleaved accumulation chains:
              #      zp[d, e*B+b] += Wd_chunk[k, d] * rt[k, b]; the host sums
              #      the two chains along with the cross-core partials ----
              zp = ps_z.tile([D1, 2 * B], f32)
              for ci in range(DCH):
                  t, o = divmod(ci, WDC)
                  e = ci % 2
                  nc.tensor.matmul(
                      zp[:, e * B:(e + 1) * B],
                      wd_tiles[t][:, o * P:(o + 1) * P],
                      rt3[:, ci, :],
                      start=(ci < 2), stop=(ci >= DCH - 2),
                  )
              o_sb = wpool.tile([D1, 2 * B], f32, tag="osb")
              nc.scalar.activation(o_sb[:], zp[:], Copy)
              nc.gpsimd.dma_start(out_ext[:], o_sb[:])

    nc.compile()
    return nc


def _make_runner_for(nc):
    return _runner_from_nc(nc)


def _get_runner(chain=1):
    """Cached jitted shard_map executable around the Bass NEFF. chain>1
    repeats the kernel body inside the NEFF for wall-clock timing."""
    key = ("runner", chain)
    if key in _cache:
        return _cache[key]
    nckey = ("nc", chain)
    nc = _cache.get(nckey)
    if nc is None:
        nc = _cache[nckey] = _build(chain)
    runner = _runner_from_nc(nc)
    _cache[key] = runner
    return runner


def _runner_from_nc(nc):
    import jax
    from jax.experimental.shard_map import shard_map
    from jax.sharding import Mesh, PartitionSpec, NamedSharding
    from concourse import bass2jax
    bass2jax.install_neuronx_cc_hook()

    partition_name = nc.partition_id_tensor.name if nc.partition_id_tensor else None
    in_names, out_names, out_avals, zero_outs = [], [], [], []
    for alloc in nc.m.functions[0].allocations:
        if not isinstance(alloc, mybir.MemoryLocationSet):
            continue
        name = alloc.memorylocations[0].name
        if alloc.kind == "ExternalInput":
            if name != partition_name:
                in_names.append(name)
        elif alloc.kind == "ExternalOutput":
            shape = tuple(alloc.tensor_shape)
            dtype = mybir.dt.np(alloc.dtype)
            out_names.append(name)
            out_avals.append(jax.core.ShapedArray(shape, dtype))
            zero_outs.append(np.zeros(shape, dtype))
    n_params = len(in_names)
    n_outs = len(out_avals)
    all_names = in_names + out_names + ([partition_name] if partition_name else [])
    donate = tuple(range(n_params, n_params + n_outs))

    def _body(*args):
        operands = list(args)
        if partition_name is not None:
            operands.append(bass2jax.partition_id_tensor())
        return tuple(bass2jax._bass_exec_p.bind(
            *operands,
            out_avals=tuple(out_avals),
            in_names=tuple(all_names),
            out_names=tuple(out_names),
            lowering_input_output_aliases=(),
            sim_require_finite=True,
            sim_require_nnan=True,
            nc=nc,
        ))

    devices = jax.devices()[:NCORES]
    mesh = Mesh(np.asarray(devices), ("core",))
    in_specs = tuple(
        PartitionSpec() if name in REPLICATED else PartitionSpec("core")
        for name in in_names) + (PartitionSpec("core"),) * n_outs
    fn = jax.jit(
        shard_map(_body, mesh=mesh, in_specs=in_specs,
                  out_specs=(PartitionSpec("core"),) * n_outs,
                  check_rep=False),
        donate_argnums=donate, keep_unused=True,
    )
    shardings = {
        name: NamedSharding(mesh, PartitionSpec() if name in REPLICATED
                            else PartitionSpec("core"))
        for name in in_names}
    return {
        "fn": fn, "in_names": in_names, "out_names": out_names,
        "zero_outs": zero_outs, "mesh": mesh,
        "sharding": NamedSharding(mesh, PartitionSpec("core")),
        "shardings": shardings,
        "out_avals": out_avals,
    }


def _prep(x, a, W1, b1, W2, b2, Wd, bd, Wo, bo):
    """Host-side shard/layout prep -> dict of concatenated (8*dim0) inputs."""
    x = np.asarray(x, np.float32)
    a = np.asarray(a, np.float32)
    W1 = np.ascontiguousarray(np.asarray(W1, NP_BF))
    W2 = np.ascontiguousarray(np.asarray(W2, NP_BF))
    b1c = np.asarray(b1, np.float32).reshape(H1, 1)
    b2c = np.asarray(b2, np.float32).reshape(H2, 1)
    Wd = np.asarray(Wd, np.float32)

    at = np.ascontiguousarray(
        a.astype(NP_F8).transpose(0, 2, 1)).reshape(NCORES * MC, P, N)
    xt = np.ascontiguousarray(
        x.astype(NP_BF).transpose(0, 2, 1)).reshape(NCORES * F, N)
    # wds[c, t, p, ci*128 + d] = Wd[c*16384 + (t*WDC + ci)*128 + p, d]
    wds = np.ascontiguousarray(
        Wd.astype(NP_BF).reshape(NCORES, WDT, WDC, P, D1)
        .transpose(0, 1, 3, 2, 4)
        .reshape(NCORES * WDT, P, WDC * P))

    return {
        "at": at, "xt": xt, "w1": W1, "w2": W2, "b1": b1c,
        "b2": b2c, "wds": wds,
    }


def _run(runner, concat_ins):
    args = [concat_ins[name] for name in runner["in_names"]]
    zeros = [np.zeros((NCORES * z.shape[0], *z.shape[1:]), z.dtype)
             for z in runner["zero_outs"]]
    return runner["fn"](*args, *zeros)


def kernel(x, a, W1, b1, W2, b2, Wd, bd, Wo, bo):
    runner = _get_runner()
    concat_ins = _prep(x, a, W1, b1, W2, b2, Wd, bd, Wo, bo)
    outs = _run(runner, concat_ins)
    oi = runner["out_names"].index("out")
    partials = np.asarray(outs[oi]).reshape(NCORES, D1, 2, B).astype(np.float64)
    z = partials.sum(axis=(0, 2)).T                # [B, D1]
    hd = np.maximum(z + np.asarray(bd, np.float64).reshape(1, D1), 0.0)
    logit = hd @ np.asarray(Wo, np.float64).reshape(D1, 1) \
        + np.asarray(bo, np.float64).reshape(1, 1)     # [B, 1]
    out = np.where(logit >= 0, 1.0 / (1.0 + np.exp(-np.clip(logit, 0, None))),
                   np.exp(np.clip(logit, None, 0))
                   / (1.0 + np.exp(np.clip(logit, None, 0))))
    return out.astype(np.float32)
